# revision 19
# baseline (speedup 1.0000x reference)
"""Causal multi-head attention on 8 Trainium2 NeuronCores.

Problem: B=2, S=2048, E=1024, H=16 heads, D=64.
Sharding: core c handles batch b = c // 4 and heads [4*(c%4) .. 4*(c%4)+3]
(data parallel on B x tensor parallel on heads). Each core computes its
partial output projection; the host sums the 4 partials per batch and adds
b_proj (the standard row-parallel TP reduction, done on host).

Per-core kernel (all matmuls bf16 inputs, fp32 PSUM accumulation), computed
in "transposed" space to avoid transposing softmax probabilities:
  xT = X[b]^T in SBUF.
  Q^T/K^T [D, S] per head via one M=128 matmul per contraction tile (the
  host packs W_q|W_k per head into adjacent 64-col groups), evicted with
  DVE tensor_scalar (bias + 1/8 scaling folded into Q).
  V [S, D] per head with a ones column appended -> AV matmul also
  accumulates softmax denominators.
  Scores S^T [k, q] = K^T.T @ Q^T, row-packed across head pairs. Causal
  masking of the 128-wide diagonal strip happens IN PSUM: an extra
  identity-lhsT matmul accumulates -30000 onto the strictly-upper strip
  (rhs = precomputed -30000 upper-tri tile), so exp gives exact zeros and
  no post-exp mask op exists on any engine.
  P^T = exp(S^T) on ACT (the only ACT work in steady state).
  The score matmuls for block j+1 are emitted before the AV matmuls of
  block j, so PE streams scores while ACT runs exp (software pipelining
  over the 2x-buffered score psum).
  A^T [65, q] = V_ext.T @ P^T accumulated over k tiles (row 64 = denom).
  Per-pair division: denominator rows -> reciprocal_approx_fast (DVE),
  broadcast across partitions via a K=2 f32r outer-product matmul,
  multiply into head-pair-stacked A_scaled^T on DVE.
  partial[q, e] = A_scaled^T.T @ W_proj_rows accumulated over both pairs,
  written back as bf16 partials (host reduces in float64).
"""

import os
import sys
from contextlib import ExitStack

for _p in ("/opt/trn_rl_repo", "/root/.axon_site/_ro/trn_rl_repo"):
    if os.path.isdir(_p) and _p not in sys.path:
        sys.path.append(_p)

import numpy as np
import ml_dtypes

import concourse.bass as bass
import concourse.tile as tile
from concourse import bacc
from concourse import mybir
from concourse.masks import make_identity

FP32 = mybir.dt.float32
F32R = mybir.dt.float32r
BF16 = mybir.dt.bfloat16
AF = mybir.ActivationFunctionType

B, S, E, H = 2, 2048, 1024, 16
D = E // H          # 64
NCORES = 8
HPC = 4             # heads per core
NPAIR = 2           # head pairs per core
KT = E // 128       # 8 contraction tiles over E
ST = S // 128       # 16 tiles over S (k dimension)
QM = S // 512       # 4 q-macro tiles of 512
NQ = 512


def build_graph():
    nc = bacc.Bacc()

    xT = nc.declare_dram_parameter("xT", [E, S], BF16, isOutput=False)
    wqk = nc.declare_dram_parameter("wqk", [E, 2 * HPC * D], BF16,
                                    isOutput=False)
    wv = nc.declare_dram_parameter("wv", [E, HPC * D], BF16, isOutput=False)
    qkbias = nc.declare_dram_parameter("qkbias", [128, HPC], FP32,
                                       isOutput=False)
    wp = nc.declare_dram_parameter("wp", [HPC * D, E], BF16, isOutput=False)
    out = nc.declare_dram_parameter("out", [S, E], BF16, isOutput=True)

    with tile.TileContext(nc) as tc, ExitStack() as ctx:
        const = ctx.enter_context(tc.tile_pool(name="const", bufs=1))
        sb = ctx.enter_context(tc.tile_pool(name="sb", bufs=1))
        pexp_pool = ctx.enter_context(tc.tile_pool(name="pexp", bufs=10))
        stage = ctx.enter_context(tc.tile_pool(name="stage", bufs=3))
        rec_pool = ctx.enter_context(tc.tile_pool(name="rec", bufs=2))
        araw_pool = ctx.enter_context(tc.tile_pool(name="araw", bufs=2))

        # PSUM budget is 8 banks: scores 2x2-bank + qkv/proj/bcast 2 + psa 2
        ps_s = ctx.enter_context(tc.tile_pool(name="ps_s", bufs=2, space="PSUM"))
        ps_qkv = ctx.enter_context(tc.tile_pool(name="ps_qkv", bufs=2, space="PSUM"))
        ps_a = ctx.enter_context(tc.tile_pool(name="ps_a", bufs=1, space="PSUM"))

        # ---- persistent SBUF tensors ----
        xt_sb = sb.tile([128, KT, S], BF16)          # X^T tiles, kt-major
        qt_sb = sb.tile([128, NPAIR, S], BF16)       # Q^T, pair-stacked
        kt_sb = sb.tile([128, NPAIR, S], BF16)       # K^T, pair-stacked
        v_sb = sb.tile([128, ST, HPC, D + 1], BF16)  # [V | ones] per ktile/head
        as_sb = sb.tile([128, NPAIR, S], BF16)       # A_scaled^T, pair-stacked
        wqk_sb = sb.tile([128, KT, 2 * HPC * D], BF16)
        wv_sb = sb.tile([128, KT, HPC * D], BF16)
        wp_sb = sb.tile([128, NPAIR, E], BF16)
        qkb_sb = const.tile([128, HPC], FP32)
        ones1 = const.tile([1, 128], BF16)           # K=1 bcast lhsT
        id128 = const.tile([128, 128], BF16)
        negtri = const.tile([128, 128], BF16)        # -30000 strictly above diag

        # ---- constants ----
        nc.any.memset(ones1[:], 1.0)
        nc.any.memset(v_sb[:, :, :, D:D + 1], 1.0)
        make_identity(nc, id128[:])
        # negtri[kk, qq] = -30000 where kk > qq else 0
        nc.any.memset(negtri[:], -30000.0)
        nc.gpsimd.affine_select(
            out=negtri[:], in_=negtri[:],
            compare_op=mybir.AluOpType.is_ge, fill=0.0,
            base=-1, pattern=[[-1, 128]], channel_multiplier=1)
        # PE warm-up: dummy matmuls on a zero tile while input DMAs land.
        warm = const.tile([128, NQ], BF16)
        nc.vector.memset(warm[:], 0.0)
        psw = ps_s.tile([128, 2 * NQ], FP32, name="psw", tag="ss")
        for _w in range(18):
            nc.tensor.matmul(psw[:, 0:NQ], lhsT=warm[:, 0:128], rhs=warm[:],
                             start=(_w == 0), stop=(_w == 17))

        # ---- input DMAs: batched 3D transfers, critical-first, both rings ----
        xT3 = xT.rearrange("(t p) s -> p t s", p=128)
        nc.sync.dma_start(xt_sb[:, :, 0:NQ], xT3[:, :, 0:NQ])
        nc.scalar.dma_start(wqk_sb[:],
                            wqk.rearrange("(t p) d -> p t d", p=128))
        nc.sync.dma_start(wv_sb[:],
                          wv.rearrange("(t p) d -> p t d", p=128))
        nc.scalar.dma_start(qkb_sb[:], qkbias[:])
        nc.sync.dma_start(xt_sb[:, :, NQ:2 * NQ], xT3[:, :, NQ:2 * NQ])
        nc.scalar.dma_start(wp_sb[:],
                            wp.rearrange("(t p) e -> p t e", p=128))
        nc.sync.dma_start(xt_sb[:, :, 2 * NQ:3 * NQ], xT3[:, :, 2 * NQ:3 * NQ])
        nc.scalar.dma_start(xt_sb[:, :, 3 * NQ:4 * NQ], xT3[:, :, 3 * NQ:4 * NQ])

        def v_unit(st):
            def emit():
                psv = ps_qkv.tile([128, NQ], FP32, name="psv", tag="qkv")
                ssl = slice(st * 128, (st + 1) * 128)
                for kt in range(KT):
                    nc.tensor.matmul(
                        psv[:, 0:HPC * D], lhsT=xt_sb[:, kt, ssl],
                        rhs=wv_sb[:, kt, :], start=(kt == 0),
                        stop=(kt == KT - 1))
                nc.vector.tensor_copy(
                    v_sb[:, st, :, 0:D],
                    psv[:, 0:HPC * D].rearrange("p (h d) -> p h d", h=HPC))
            return emit

        def qk_unit(mm, h):
            def emit():
                msl = slice(mm * NQ, (mm + 1) * NQ)
                p, hh = h // 2, h % 2
                lo, hi = hh * 64, hh * 64 + 64
                psqk = ps_qkv.tile([128, NQ], FP32, name="psqk", tag="qkv")
                for kt in range(KT):
                    nc.tensor.matmul(
                        psqk[:],
                        lhsT=wqk_sb[:, kt, h * 128:(h + 1) * 128],
                        rhs=xt_sb[:, kt, msl],
                        start=(kt == 0), stop=(kt == KT - 1))
                nc.vector.tensor_scalar(
                    qt_sb[lo:hi, p, msl], psqk[0:64, :],
                    0.125, qkb_sb[0:64, h:h + 1],
                    op0=mybir.AluOpType.mult, op1=mybir.AluOpType.add)
                nc.vector.tensor_scalar_add(
                    kt_sb[lo:hi, p, msl], psqk[64:128, :],
                    qkb_sb[64:128, h:h + 1])
            return emit

        def qkv_units(mm):
            us = [v_unit(st) for st in range(4 * mm, 4 * mm + 4)]
            us += [qk_unit(mm, h) for h in range(HPC)]
            return us

        # macro 0 startup: pair-0 QK first so its scores begin while the
        # remaining input DMAs land, then V (needed by the first AVs)
        for u in ([qk_unit(0, 0), qk_unit(0, 1)]
                  + [v_unit(st) for st in range(4)]
                  + [qk_unit(0, 2), qk_unit(0, 3)]):
            u()
        proj_stash = []
        dmaq = [nc.sync, nc.scalar]
        for m in range(QM):
            msl = slice(m * NQ, (m + 1) * NQ)
            nblk = 4 * m + 4
            coll = [rec_pool.tile([1, 2 * NQ], FP32, name=f"coll{_p}",
                                  tag=f"coll{_p}") for _p in range(NPAIR)]
            recf = [rec_pool.tile([1, 2 * NQ], FP32, name=f"recf{_p}",
                                  tag=f"recf{_p}") for _p in range(NPAIR)]
            recb = [rec_pool.tile([1, 2 * NQ], BF16, name=f"recb{_p}",
                                  tag=f"recb{_p}") for _p in range(NPAIR)]
            araw = [araw_pool.tile([64, NQ], BF16, name=f"araw{_h}",
                                   tag=f"araw{_h}") for _h in range(HPC)]

            def proj_unit(mm, t):
                def emit():
                    tsl = slice(mm * NQ + t * 128, mm * NQ + (t + 1) * 128)
                    osb = stage.tile([128, E], BF16)
                    for e in range(2):
                        esl = slice(e * 512, (e + 1) * 512)
                        pso = ps_qkv.tile([128, 512], FP32, name="pso",
                                          tag="qkv")
                        nc.tensor.matmul(
                            pso[:], lhsT=as_sb[:, 0, tsl],
                            rhs=wp_sb[:, 0, esl], start=True, stop=False)
                        nc.tensor.matmul(
                            pso[:], lhsT=as_sb[:, 1, tsl],
                            rhs=wp_sb[:, 1, esl], start=False, stop=True)
                        nc.vector.tensor_copy(osb[:, esl], pso[:])
                    dmaq[t % 2].dma_start(out[tsl, :], osb[:])
                return emit

            filler = qkv_units(m + 1) if m + 1 < QM else []
            for p in range(NPAIR):
                if p == 1 and proj_stash:
                    # previous macro's projection: pure PE filler at the seam
                    while proj_stash:
                        proj_stash.pop(0)()
                psa = [ps_a.tile([65, NQ], FP32, name=f"psa{_hh}",
                                 tag=f"psa{_hh}")
                       for _hh in range(2)]

                def emit_scores(j):
                    r = j - 4 * m
                    c0 = 128 * r if r > 0 else 0
                    jsl = slice(j * 128, (j + 1) * 128)
                    pss = ps_s.tile([128, 2 * NQ], FP32, name="pss", tag="ss")
                    for hh in range(2):
                        lo, hi = hh * 64, hh * 64 + 64
                        nc.tensor.matmul(
                            pss[:, hh * NQ + c0:hh * NQ + NQ],
                            lhsT=kt_sb[lo:hi, p, jsl],
                            rhs=qt_sb[lo:hi, p, m * NQ + c0:m * NQ + NQ],
                            start=True, stop=(r < 0),
                            tile_position=(hh * 64, 0))
                    if r >= 0:
                        # causal mask in psum: accumulate -30000 onto the
                        # strictly-upper 128-wide strip so exp zeroes it
                        for hh in range(2):
                            nc.tensor.matmul(
                                pss[:, hh * NQ + c0:hh * NQ + c0 + 128],
                                lhsT=id128[:], rhs=negtri[:],
                                start=False, stop=True)
                    return pss, c0

                cur = emit_scores(0)
                for j in range(nblk):
                    nxt = emit_scores(j + 1) if j + 1 < nblk else None
                    pss, c0 = cur
                    pe = pexp_pool.tile([128, 2 * NQ], BF16)
                    # one exp for both heads (halves ACT per-op overhead)
                    nc.scalar.activation(
                        pe[:].rearrange("p (g q) -> p g q", g=2)[:, :, c0:NQ],
                        pss[:].rearrange("p (g q) -> p g q", g=2)[:, :, c0:NQ],
                        AF.Exp)
                    for hh in range(2):
                        nc.tensor.matmul(
                            psa[hh][:, c0:NQ],
                            lhsT=v_sb[:, j, 2 * p + hh, :],
                            rhs=pe[:, hh * NQ + c0:hh * NQ + NQ],
                            start=(j == 0), stop=(j == nblk - 1))
                    cur = nxt
                # per-pair division: denom rows -> fast reciprocal ->
                # partition-broadcast matmul (f32r) -> scale into as_sb
                for hh in range(2):
                    nc.vector.tensor_copy(
                        coll[p][0:1, hh * NQ:(hh + 1) * NQ],
                        psa[hh][64:65, :])
                nc.vector.reciprocal_approx_fast(
                    out=recf[p][:], in_=coll[p][:])
                nc.vector.tensor_copy(recb[p][:], recf[p][:])
                nc.scalar.copy(araw[2 * p][:], psa[0][0:64, :])
                nc.vector.tensor_copy(araw[2 * p + 1][:], psa[1][0:64, :])
                psb = ps_qkv.tile([128, NQ], FP32, name="psb", tag="qkv")
                for hh in range(2):
                    nc.tensor.matmul(
                        psb[hh * 64:(hh + 1) * 64, :],
                        lhsT=ones1[0:1, hh * 64:(hh + 1) * 64],
                        rhs=recb[p][0:1, hh * NQ:(hh + 1) * NQ],
                        start=True, stop=True,
                        tile_position=(0, hh * 64), skip_group_check=True)
                for hh in range(2):
                    lo, hi = hh * 64, hh * 64 + 64
                    nc.vector.tensor_mul(
                        as_sb[lo:hi, p, msl], psb[lo:hi, :],
                        araw[2 * p + hh][:])
            for u in filler:
                u()
            # projection units: stashed and emitted at the next macro's
            # pair seam (pure filler); last macro emitted immediately
            for t in range(4):
                if m + 1 < QM:
                    proj_stash.append(proj_unit(m, t))
                else:
                    proj_unit(m, t)()

    nc.compile()
    return nc


_CACHED = {}


def _get_graph():
    if "nc" not in _CACHED:
        _CACHED["nc"] = build_graph()
    return _CACHED["nc"]


def make_in_maps(hidden_states, W_qkv, b_qkv, W_proj):
    bf16 = ml_dtypes.bfloat16
    in_maps = []
    xTb = [np.ascontiguousarray(hidden_states[b].T).astype(bf16)
           for b in range(B)]
    for c in range(NCORES):
        b = c // 4
        h0 = HPC * (c % 4)
        csl = slice(h0 * D, (h0 + HPC) * D)
        wq_s = W_qkv[:, csl]
        wk_s = W_qkv[:, E:][:, csl]
        # per head h: [W_q cols | W_k cols] packed into one 128-col group
        wqk_s = np.empty((E, 2 * HPC * D), dtype=np.float32)
        for h in range(HPC):
            wqk_s[:, h * 128:h * 128 + 64] = wq_s[:, h * 64:(h + 1) * 64]
            wqk_s[:, h * 128 + 64:(h + 1) * 128] = wk_s[:, h * 64:(h + 1) * 64]
        wqk_s = np.ascontiguousarray(wqk_s).astype(bf16)
        wv_s = np.ascontiguousarray(W_qkv[:, 2 * E:][:, csl]).astype(bf16)
        bq = b_qkv[csl].reshape(HPC, D).T.astype(np.float32) / 8.0
        bk = b_qkv[E:][csl].reshape(HPC, D).T.astype(np.float32)
        qkbias = np.ascontiguousarray(
            np.concatenate([bq, bk], axis=0))          # (128, 4)
        wp_s = np.ascontiguousarray(W_proj[csl, :]).astype(bf16)
        in_maps.append({
            "xT": xTb[b], "wqk": wqk_s, "wv": wv_s,
            "qkbias": qkbias, "wp": wp_s,
        })
    return in_maps


def kernel(hidden_states, W_qkv, b_qkv, W_proj, b_proj):
    from concourse.bass_utils import run_bass_kernel_spmd

    hidden_states = np.asarray(hidden_states, dtype=np.float32)
    W_qkv = np.asarray(W_qkv, dtype=np.float32)
    b_qkv = np.asarray(b_qkv, dtype=np.float32)
    W_proj = np.asarray(W_proj, dtype=np.float32)
    b_proj = np.asarray(b_proj, dtype=np.float32)

    nc = _get_graph()
    in_maps = make_in_maps(hidden_states, W_qkv, b_qkv, W_proj)
    res = None
    for attempt in range(3):
        try:
            res = run_bass_kernel_spmd(nc, in_maps, list(range(NCORES)))
            break
        except Exception:
            if attempt == 2:
                raise
            import time
            time.sleep(30 * (attempt + 1))
    partials = [res.results[c]["out"] for c in range(NCORES)]
    # V-bias folded out of the device kernel: softmax rows sum to one, so
    # the bias contributes bv @ W_proj to every output row exactly once.
    bias_eff = b_proj.astype(np.float64) + (
        b_qkv[2 * E:].astype(np.float64) @ W_proj.astype(np.float64))
    outp = np.empty((B, S, E), dtype=np.float32)
    for b in range(B):
        acc = np.zeros((S, E), dtype=np.float64)
        for c in range(4 * b, 4 * b + 4):
            acc += partials[c].astype(np.float64)
        outp[b] = (acc + bias_eff).astype(np.float32)
    return outp


# revision 25
# speedup vs baseline: 1.0848x; 1.0848x over previous
"""Causal multi-head attention on 8 Trainium2 NeuronCores.

Problem: B=2, S=2048, E=1024, H=16 heads, D=64.
Sharding: core c handles batch b = c // 4 and heads [4*(c%4) .. 4*(c%4)+3]
(data parallel on B x tensor parallel on heads). Each core computes its
partial output projection; the host sums the 4 partials per batch and adds
b_proj (the standard row-parallel TP reduction, done on host).

Per-core kernel (all matmuls bf16 inputs, fp32 PSUM accumulation), computed
in "transposed" space to avoid transposing softmax probabilities:
  xT = X[b]^T in SBUF.
  Q^T/K^T [D, S] per head via one M=128 matmul per contraction tile (the
  host packs W_q|W_k per head into adjacent 64-col groups), evicted with
  DVE tensor_scalar (bias + 1/8 scaling folded into Q).
  V [S, D] per head with a ones column appended -> AV matmul also
  accumulates softmax denominators.
  Scores S^T [k, q] = K^T.T @ Q^T, row-packed across head pairs. Causal
  masking of the 128-wide diagonal strip happens IN PSUM: an extra
  identity-lhsT matmul accumulates -30000 onto the strictly-upper strip
  (rhs = precomputed -30000 upper-tri tile), so exp gives exact zeros and
  no post-exp mask op exists on any engine.
  P^T = exp(S^T) on ACT (the only ACT work in steady state).
  The score matmuls for block j+1 are emitted before the AV matmuls of
  block j, so PE streams scores while ACT runs exp (software pipelining
  over the 2x-buffered score psum).
  A^T [65, q] = V_ext.T @ P^T accumulated over k tiles (row 64 = denom).
  Per-pair division: denominator rows -> reciprocal_approx_fast (DVE),
  broadcast across partitions via a K=2 f32r outer-product matmul,
  multiply into head-pair-stacked A_scaled^T on DVE.
  partial[q, e] = A_scaled^T.T @ W_proj_rows accumulated over both pairs,
  written back as bf16 partials (host reduces in float64).
"""

import os
import sys
from contextlib import ExitStack

for _p in ("/opt/trn_rl_repo", "/root/.axon_site/_ro/trn_rl_repo"):
    if os.path.isdir(_p) and _p not in sys.path:
        sys.path.append(_p)

import numpy as np
import ml_dtypes

import concourse.bass as bass
import concourse.tile as tile
from concourse import bacc
from concourse import mybir
from concourse.masks import make_identity

FP32 = mybir.dt.float32
F32R = mybir.dt.float32r
BF16 = mybir.dt.bfloat16
AF = mybir.ActivationFunctionType

B, S, E, H = 2, 2048, 1024, 16
D = E // H          # 64
NCORES = 8
HPC = 4             # heads per core
NPAIR = 2           # head pairs per core
KT = E // 128       # 8 contraction tiles over E
ST = S // 128       # 16 tiles over S (k dimension)
QM = S // 512       # 4 q-macro tiles of 512
NQ = 512


def build_graph():
    nc = bacc.Bacc()

    xT = nc.declare_dram_parameter("xT", [E, S], BF16, isOutput=False)
    wqk = nc.declare_dram_parameter("wqk", [E, 2 * HPC * D], BF16,
                                    isOutput=False)
    wv = nc.declare_dram_parameter("wv", [E, HPC * D], BF16, isOutput=False)
    qkbias = nc.declare_dram_parameter("qkbias", [128, HPC], FP32,
                                       isOutput=False)
    wp = nc.declare_dram_parameter("wp", [HPC * D, E], BF16, isOutput=False)
    out = nc.declare_dram_parameter("out", [S, E], BF16, isOutput=True)

    with tile.TileContext(nc) as tc, ExitStack() as ctx:
        const = ctx.enter_context(tc.tile_pool(name="const", bufs=1))
        sb = ctx.enter_context(tc.tile_pool(name="sb", bufs=1))
        pexp_pool = ctx.enter_context(tc.tile_pool(name="pexp", bufs=10))
        stage = ctx.enter_context(tc.tile_pool(name="stage", bufs=3))
        rec_pool = ctx.enter_context(tc.tile_pool(name="rec", bufs=2))
        araw_pool = ctx.enter_context(tc.tile_pool(name="araw", bufs=2))

        # PSUM budget is 8 banks: scores 2x2-bank + qkv/proj/bcast 2 + psa 2
        ps_s = ctx.enter_context(tc.tile_pool(name="ps_s", bufs=2, space="PSUM"))
        ps_qkv = ctx.enter_context(tc.tile_pool(name="ps_qkv", bufs=2, space="PSUM"))
        ps_a = ctx.enter_context(tc.tile_pool(name="ps_a", bufs=1, space="PSUM"))

        # ---- persistent SBUF tensors ----
        xt_sb = sb.tile([128, KT, S], BF16)          # X^T tiles, kt-major
        qt_sb = sb.tile([128, NPAIR, S], BF16)       # Q^T, pair-stacked
        kt_sb = sb.tile([128, NPAIR, S], BF16)       # K^T, pair-stacked
        v_sb = sb.tile([128, ST, HPC, D + 1], BF16)  # [V | ones] per ktile/head
        as_sb = sb.tile([128, NPAIR, S], BF16)       # A_scaled^T, pair-stacked
        wqk_sb = sb.tile([128, KT, 2 * HPC * D], BF16)
        wv_sb = sb.tile([128, KT, HPC * D], BF16)
        wp_sb = sb.tile([128, NPAIR, E], BF16)
        qkb_sb = const.tile([128, HPC], FP32)
        ones1 = const.tile([1, 128], BF16)           # K=1 bcast lhsT
        id128 = const.tile([128, 128], BF16)
        negtri = const.tile([128, 128], BF16)        # -30000 strictly above diag

        # ---- constants ----
        nc.any.memset(ones1[:], 1.0)
        nc.any.memset(v_sb[:, :, :, D:D + 1], 1.0)
        make_identity(nc, id128[:])
        # negtri[kk, qq] = -30000 where kk > qq else 0
        nc.any.memset(negtri[:], -30000.0)
        nc.gpsimd.affine_select(
            out=negtri[:], in_=negtri[:],
            compare_op=mybir.AluOpType.is_ge, fill=0.0,
            base=-1, pattern=[[-1, 128]], channel_multiplier=1)
        # PE warm-up: dummy matmuls on a zero tile while input DMAs land.
        warm = const.tile([128, NQ], BF16)
        nc.vector.memset(warm[:], 0.0)
        psw = ps_s.tile([128, 2 * NQ], FP32, name="psw", tag="ss")
        for _w in range(18):
            nc.tensor.matmul(psw[:, 0:NQ], lhsT=warm[:, 0:128], rhs=warm[:],
                             start=(_w == 0), stop=(_w == 17))

        # ---- input DMAs: batched 3D transfers, critical-first, both rings ----
        xT3 = xT.rearrange("(t p) s -> p t s", p=128)
        wqk3 = wqk.rearrange("(t p) d -> p t d", p=128)
        nc.sync.dma_start(xt_sb[:, 0:4, 0:NQ], xT3[:, 0:4, 0:NQ])
        nc.scalar.dma_start(wqk_sb[:, 0:4, :], wqk3[:, 0:4, :])
        nc.sync.dma_start(xt_sb[:, 4:8, 0:NQ], xT3[:, 4:8, 0:NQ])
        nc.scalar.dma_start(wqk_sb[:, 4:8, :], wqk3[:, 4:8, :])
        nc.sync.dma_start(wv_sb[:],
                          wv.rearrange("(t p) d -> p t d", p=128))
        nc.scalar.dma_start(qkb_sb[:], qkbias[:])
        nc.sync.dma_start(xt_sb[:, :, NQ:2 * NQ], xT3[:, :, NQ:2 * NQ])
        nc.scalar.dma_start(wp_sb[:],
                            wp.rearrange("(t p) e -> p t e", p=128))
        nc.sync.dma_start(xt_sb[:, :, 2 * NQ:3 * NQ], xT3[:, :, 2 * NQ:3 * NQ])
        nc.scalar.dma_start(xt_sb[:, :, 3 * NQ:4 * NQ], xT3[:, :, 3 * NQ:4 * NQ])

        def v_unit(st):
            def emit():
                psv = ps_qkv.tile([128, NQ], FP32, name="psv", tag="qkv")
                ssl = slice(st * 128, (st + 1) * 128)
                for kt in range(KT):
                    nc.tensor.matmul(
                        psv[:, 0:HPC * D], lhsT=xt_sb[:, kt, ssl],
                        rhs=wv_sb[:, kt, :], start=(kt == 0),
                        stop=(kt == KT - 1))
                nc.vector.tensor_copy(
                    v_sb[:, st, :, 0:D],
                    psv[:, 0:HPC * D].rearrange("p (h d) -> p h d", h=HPC))
            return emit

        def qk_unit(mm, h):
            def emit():
                msl = slice(mm * NQ, (mm + 1) * NQ)
                p, hh = h // 2, h % 2
                lo, hi = hh * 64, hh * 64 + 64
                psqk = ps_qkv.tile([128, NQ], FP32, name="psqk", tag="qkv")
                for kt in range(KT):
                    nc.tensor.matmul(
                        psqk[:],
                        lhsT=wqk_sb[:, kt, h * 128:(h + 1) * 128],
                        rhs=xt_sb[:, kt, msl],
                        start=(kt == 0), stop=(kt == KT - 1))
                nc.vector.tensor_scalar(
                    qt_sb[lo:hi, p, msl], psqk[0:64, :],
                    0.125, qkb_sb[0:64, h:h + 1],
                    op0=mybir.AluOpType.mult, op1=mybir.AluOpType.add)
                nc.vector.tensor_scalar_add(
                    kt_sb[lo:hi, p, msl], psqk[64:128, :],
                    qkb_sb[64:128, h:h + 1])
            return emit

        def qkv_units(mm):
            us = [v_unit(st) for st in range(4 * mm, 4 * mm + 4)]
            us += [qk_unit(mm, h) for h in range(HPC)]
            return us

        # macro 0 startup: pair-0 QK first so its scores begin while the
        # remaining input DMAs land, then V (needed by the first AVs)
        for u in ([qk_unit(0, 0), qk_unit(0, 1)]
                  + [v_unit(st) for st in range(4)]
                  + [qk_unit(0, 2), qk_unit(0, 3)]):
            u()
        proj_stash = []
        dmaq = [nc.sync, nc.scalar]
        for m in range(QM):
            msl = slice(m * NQ, (m + 1) * NQ)
            nblk = 4 * m + 4
            coll = [rec_pool.tile([1, 2 * NQ], FP32, name=f"coll{_p}",
                                  tag=f"coll{_p}") for _p in range(NPAIR)]
            recf = [rec_pool.tile([1, 2 * NQ], FP32, name=f"recf{_p}",
                                  tag=f"recf{_p}") for _p in range(NPAIR)]
            recb = [rec_pool.tile([1, 2 * NQ], BF16, name=f"recb{_p}",
                                  tag=f"recb{_p}") for _p in range(NPAIR)]
            araw = [araw_pool.tile([64, NQ], BF16, name=f"araw{_h}",
                                   tag=f"araw{_h}") for _h in range(HPC)]

            def proj_unit(mm, t, evict_act=False):
                def emit():
                    tsl = slice(mm * NQ + t * 128, mm * NQ + (t + 1) * 128)
                    osb = stage.tile([128, E], BF16)
                    for e in range(2):
                        esl = slice(e * 512, (e + 1) * 512)
                        pso = ps_qkv.tile([128, 512], FP32, name="pso",
                                          tag="qkv")
                        nc.tensor.matmul(
                            pso[:], lhsT=as_sb[:, 0, tsl],
                            rhs=wp_sb[:, 0, esl], start=True, stop=False)
                        nc.tensor.matmul(
                            pso[:], lhsT=as_sb[:, 1, tsl],
                            rhs=wp_sb[:, 1, esl], start=False, stop=True)
                        if evict_act:
                            nc.scalar.copy(osb[:, esl], pso[:])
                        else:
                            nc.vector.tensor_copy(osb[:, esl], pso[:])
                    dmaq[t % 2].dma_start(out[tsl, :], osb[:])
                return emit

            filler = qkv_units(m + 1) if m + 1 < QM else []
            div_b_stash = []

            def division_b(p_, psa_):
                # recb on ACT (idle while the seam's proj filler runs on PE),
                # broadcast matmuls on PE, scale-muls on DVE
                def emit():
                    nc.scalar.copy(recb[p_][:], recf[p_][:])
                    psb = ps_qkv.tile([128, NQ], FP32, name="psb", tag="qkv")
                    for hh in range(2):
                        nc.tensor.matmul(
                            psb[hh * 64:(hh + 1) * 64, :],
                            lhsT=ones1[0:1, hh * 64:(hh + 1) * 64],
                            rhs=recb[p_][0:1, hh * NQ:(hh + 1) * NQ],
                            start=True, stop=True,
                            tile_position=(0, hh * 64), skip_group_check=True)
                    for hh in range(2):
                        lo, hi = hh * 64, hh * 64 + 64
                        nc.vector.tensor_mul(
                            as_sb[lo:hi, p_, msl], psb[lo:hi, :],
                            araw[2 * p_ + hh][:])
                return emit

            for p in range(NPAIR):
                if p == 1 and proj_stash:
                    # previous macro's projection: pure PE filler at the seam
                    while proj_stash:
                        proj_stash.pop(0)()
                psa = [ps_a.tile([65, NQ], FP32, name=f"psa{_hh}",
                                 tag=f"psa{_hh}")
                       for _hh in range(2)]

                def emit_scores(j):
                    r = j - 4 * m
                    c0 = 128 * r if r > 0 else 0
                    jsl = slice(j * 128, (j + 1) * 128)
                    pss = ps_s.tile([128, 2 * NQ], FP32, name="pss", tag="ss")
                    for hh in range(2):
                        lo, hi = hh * 64, hh * 64 + 64
                        nc.tensor.matmul(
                            pss[:, hh * NQ + c0:hh * NQ + NQ],
                            lhsT=kt_sb[lo:hi, p, jsl],
                            rhs=qt_sb[lo:hi, p, m * NQ + c0:m * NQ + NQ],
                            start=True, stop=(r < 0),
                            tile_position=(hh * 64, 0))
                    if r >= 0:
                        # causal mask in psum: accumulate -30000 onto the
                        # strictly-upper 128-wide strip so exp zeroes it
                        for hh in range(2):
                            nc.tensor.matmul(
                                pss[:, hh * NQ + c0:hh * NQ + c0 + 128],
                                lhsT=id128[:], rhs=negtri[:],
                                start=False, stop=True)
                    return pss, c0

                flush_at = 2 if m == 0 else 1
                cur = emit_scores(0)
                for j in range(nblk):
                    nxt = emit_scores(j + 1) if j + 1 < nblk else None
                    pss, c0 = cur
                    pe = pexp_pool.tile([128, 2 * NQ], BF16)
                    # one exp for both heads (halves ACT per-op overhead)
                    nc.scalar.activation(
                        pe[:].rearrange("p (g q) -> p g q", g=2)[:, :, c0:NQ],
                        pss[:].rearrange("p (g q) -> p g q", g=2)[:, :, c0:NQ],
                        AF.Exp)
                    for hh in range(2):
                        nc.tensor.matmul(
                            psa[hh][:, c0:NQ],
                            lhsT=v_sb[:, j, 2 * p + hh, :],
                            rhs=pe[:, hh * NQ + c0:hh * NQ + NQ],
                            start=(j == 0), stop=(j == nblk - 1))
                    if j == flush_at:
                        # previous pair's broadcast+scale: deps are met by
                        # now, so these PE/DVE ops slot in without stalling
                        while div_b_stash:
                            div_b_stash.pop(0)()
                    cur = nxt
                # division phase A: DVE/ACT only (no PE ops at the seam
                # head); phase B is stashed until the next pair is rolling
                last = (m + 1 == QM and p == 1)
                nc.scalar.copy(coll[p][0:1, 0:NQ], psa[0][64:65, :])
                nc.vector.tensor_copy(coll[p][0:1, NQ:2 * NQ],
                                      psa[1][64:65, :])
                nc.vector.reciprocal_approx_fast(
                    out=recf[p][:], in_=coll[p][:])
                nc.scalar.copy(araw[2 * p][:], psa[0][0:64, :])
                nc.vector.tensor_copy(araw[2 * p + 1][:], psa[1][0:64, :])
                if not last:
                    div_b_stash.append(division_b(p, psa))
            if filler:
                filler[0]()
                filler[1]()
                filler[2]()
                while div_b_stash:
                    div_b_stash.pop(0)()
                for u in filler[3:]:
                    u()
                for t in range(4):
                    proj_stash.append(proj_unit(m, t))
            else:
                # last macro tail: chunked division straight out of psum
                # (no araw copy), interleaved with the projection units so
                # PE/ACT/DVE pipeline; output eviction on the idle ACT
                nc.scalar.copy(recb[1][:], recf[1][:])
                for t in range(4):
                    ccs = slice(t * 128, (t + 1) * 128)
                    psb = ps_qkv.tile([128, NQ], FP32, name="psb", tag="qkv")
                    for hh in range(2):
                        nc.tensor.matmul(
                            psb[hh * 64:(hh + 1) * 64, ccs],
                            lhsT=ones1[0:1, hh * 64:(hh + 1) * 64],
                            rhs=recb[1][0:1, hh * NQ + t * 128:
                                        hh * NQ + (t + 1) * 128],
                            start=True, stop=True,
                            tile_position=(0, hh * 64), skip_group_check=True)
                    for hh in range(2):
                        lo, hi = hh * 64, hh * 64 + 64
                        nc.vector.tensor_mul(
                            as_sb[lo:hi, 1, m * NQ + t * 128:
                                  m * NQ + (t + 1) * 128],
                            psb[lo:hi, ccs], araw[2 + hh][:, ccs])
                    proj_unit(m, t, evict_act=True)()

    nc.compile()
    return nc


_CACHED = {}


def _get_graph():
    if "nc" not in _CACHED:
        _CACHED["nc"] = build_graph()
    return _CACHED["nc"]


def make_in_maps(hidden_states, W_qkv, b_qkv, W_proj):
    bf16 = ml_dtypes.bfloat16
    in_maps = []
    xTb = [np.ascontiguousarray(hidden_states[b].T).astype(bf16)
           for b in range(B)]
    for c in range(NCORES):
        b = c // 4
        h0 = HPC * (c % 4)
        csl = slice(h0 * D, (h0 + HPC) * D)
        wq_s = W_qkv[:, csl]
        wk_s = W_qkv[:, E:][:, csl]
        # per head h: [W_q cols | W_k cols] packed into one 128-col group
        wqk_s = np.empty((E, 2 * HPC * D), dtype=np.float32)
        for h in range(HPC):
            wqk_s[:, h * 128:h * 128 + 64] = wq_s[:, h * 64:(h + 1) * 64]
            wqk_s[:, h * 128 + 64:(h + 1) * 128] = wk_s[:, h * 64:(h + 1) * 64]
        wqk_s = np.ascontiguousarray(wqk_s).astype(bf16)
        wv_s = np.ascontiguousarray(W_qkv[:, 2 * E:][:, csl]).astype(bf16)
        bq = b_qkv[csl].reshape(HPC, D).T.astype(np.float32) / 8.0
        bk = b_qkv[E:][csl].reshape(HPC, D).T.astype(np.float32)
        qkbias = np.ascontiguousarray(
            np.concatenate([bq, bk], axis=0))          # (128, 4)
        wp_s = np.ascontiguousarray(W_proj[csl, :]).astype(bf16)
        in_maps.append({
            "xT": xTb[b], "wqk": wqk_s, "wv": wv_s,
            "qkbias": qkbias, "wp": wp_s,
        })
    return in_maps


def kernel(hidden_states, W_qkv, b_qkv, W_proj, b_proj):
    from concourse.bass_utils import run_bass_kernel_spmd

    hidden_states = np.asarray(hidden_states, dtype=np.float32)
    W_qkv = np.asarray(W_qkv, dtype=np.float32)
    b_qkv = np.asarray(b_qkv, dtype=np.float32)
    W_proj = np.asarray(W_proj, dtype=np.float32)
    b_proj = np.asarray(b_proj, dtype=np.float32)

    nc = _get_graph()
    in_maps = make_in_maps(hidden_states, W_qkv, b_qkv, W_proj)
    res = None
    for attempt in range(3):
        try:
            res = run_bass_kernel_spmd(nc, in_maps, list(range(NCORES)))
            break
        except Exception:
            if attempt == 2:
                raise
            import time
            time.sleep(30 * (attempt + 1))
    partials = [res.results[c]["out"] for c in range(NCORES)]
    # V-bias folded out of the device kernel: softmax rows sum to one, so
    # the bias contributes bv @ W_proj to every output row exactly once.
    bias_eff = b_proj.astype(np.float64) + (
        b_qkv[2 * E:].astype(np.float64) @ W_proj.astype(np.float64))
    outp = np.empty((B, S, E), dtype=np.float32)
    for b in range(B):
        acc = np.zeros((S, E), dtype=np.float64)
        for c in range(4 * b, 4 * b + 4):
            acc += partials[c].astype(np.float64)
        outp[b] = (acc + bias_eff).astype(np.float32)
    return outp
